# revision 1
# baseline (speedup 1.0000x reference)
"""AttnBlock (GroupNorm -> 1x1 qkv conv -> full attention -> 1x1 proj -> residual)
for x[8, 256, 64, 64] fp32, data-parallel over batch on 8 NeuronCores.

Per-core plan (one image, c=256 channels on 2x128 partitions, n=4096 tokens):
  - GroupNorm(32 groups of 8 channels): per-channel bn_stats/bn_aggr along the
    free axis, then tiny fp32 indicator matmuls to reduce/broadcast across the
    8-partition channel groups; normalization folded to h = x*alpha + beta.
  - QKV: fp32r matmuls against pre-transposed weights (host-side numpy prep).
    The 1/sqrt(c) score scale is folded into W_q/b_q on the host. V is produced
    directly transposed (Vt[n, c] = h^T @ Wv^T) so the PV matmul needs no
    transposes.
  - Attention, per 512-query block: S^T[k,q] = K^T Q via PE (keys on
    partitions), exp on ACT straight out of PSUM (no max subtraction needed:
    |scores| <~ 8 so exp is safely in fp32 range), Z accumulated on DVE,
    PV accumulated over 32 key chunks in PSUM, then scaled by 1/Z (ones-matmul
    partition broadcast). The S pipeline runs 3 chunks ahead of PV, and the
    previous block's epilogue (Z reduce/broadcast/scale) plus its proj matmuls
    are injected into the next block's PE stream so the in-order PE never
    stalls on the cross-engine softmax-normalizer chain.
  - proj bias on ACT; the residual comes from pre-filling the output DRAM with
    x and accumulating proj results via DMA accum_op=add (last block uses a
    fused DVE op + plain DMA to shorten the final serial tail).
"""

import contextlib
import ctypes
import os
import sys
import types

import numpy as np

import concourse.tile as tile
from concourse import bacc, mybir
from concourse.bass_utils import run_bass_kernel_spmd


def _ensure_ntff_hook() -> bool:
    """Install an antenv.axon_hooks shim backed by libaxon_pjrt.so so that
    run_bass_kernel_spmd(trace=True) can capture NTFF profiles under axon.
    Returns True when tracing is possible."""
    try:
        from antenv.axon_hooks import get_axon_ntff_profile_hook  # noqa: F401

        return True
    except ImportError:
        pass
    so_path = "/opt/axon/libaxon_pjrt.so"
    if not os.path.exists(so_path):
        return False
    try:
        lib = ctypes.CDLL(so_path)
        if not hasattr(lib, "axon_start_nrt_profile"):
            return False
        lib.axon_start_nrt_profile.argtypes = [
            ctypes.POINTER(ctypes.c_int64),
            ctypes.c_size_t,
        ]
        lib.axon_start_nrt_profile.restype = ctypes.c_int64
        lib.axon_stop_nrt_profile.argtypes = [ctypes.c_char_p]
        lib.axon_stop_nrt_profile.restype = ctypes.c_int64
    except OSError:
        return False

    @contextlib.contextmanager
    def _hook(output_dir, device_ids):
        import jax

        jax.devices()
        if device_ids:
            ids = (ctypes.c_int64 * len(device_ids))(*device_ids)
            rc = lib.axon_start_nrt_profile(ids, len(device_ids))
        else:
            rc = lib.axon_start_nrt_profile(None, 0)
        if rc != 0:
            raise RuntimeError(f"axon_start_nrt_profile rc={rc}")
        try:
            yield
        finally:
            n = lib.axon_stop_nrt_profile(str(output_dir).encode())
            print(f"profile: {n} file(s) written to {output_dir}", file=sys.stderr)

    mod = types.ModuleType("antenv.axon_hooks")
    _state = {"hook": _hook}
    mod.get_axon_ntff_profile_hook = lambda: _state["hook"]
    mod.set_axon_ntff_profile_hook = lambda h: _state.__setitem__("hook", h)
    sys.modules["antenv.axon_hooks"] = mod
    import antenv

    antenv.axon_hooks = mod
    return True

F32 = mybir.dt.float32
F32R = mybir.dt.float32r
AX = mybir.AluOpType
AF = mybir.ActivationFunctionType

C = 256          # channels
N = 4096         # tokens (64*64)
P = 128          # partitions
CO = 2           # channel chunks (C // P)
QB = 512         # queries per block
NQB = N // QB    # 8 query blocks
NKC = N // P     # 32 key chunks
EPS = 1e-5

_LAST_RESULTS = None


def _build_program():
    nc = bacc.Bacc("TRN2", target_bir_lowering=False, debug=False, num_devices=8)

    x_d = nc.dram_tensor("x", [C, N], F32, kind="ExternalInput").ap()
    wqkT_d = nc.dram_tensor("wqkT", [C, 3 * C], F32R, kind="ExternalInput").ap()
    bq_d = nc.dram_tensor("bq", [C], F32, kind="ExternalInput").ap()
    bk_d = nc.dram_tensor("bk", [C], F32, kind="ExternalInput").ap()
    bv_d = nc.dram_tensor("bv", [C], F32R, kind="ExternalInput").ap()
    projT_d = nc.dram_tensor("projT", [C, C], F32R, kind="ExternalInput").ap()
    pb_d = nc.dram_tensor("pb", [C], F32, kind="ExternalInput").ap()
    nw_d = nc.dram_tensor("nw", [C], F32, kind="ExternalInput").ap()
    nb_d = nc.dram_tensor("nb", [C], F32, kind="ExternalInput").ap()
    gh_d = nc.dram_tensor("ghmat", [P, P], F32, kind="ExternalInput").ap()
    ones128_d = nc.dram_tensor("ones128", [P, 1], F32R, kind="ExternalInput").ap()
    ones1_d = nc.dram_tensor("ones1", [1, P], F32R, kind="ExternalInput").ap()
    out_d = nc.dram_tensor("out", [C, N], F32, kind="ExternalOutput").ap()

    # channel c = o*128 + p  ->  [partition, chunk, free]
    x_v = x_d.rearrange("(o p) m -> p o m", p=P)
    wqkT_v = wqkT_d.rearrange("(o p) m -> p o m", p=P)
    projT_v = projT_d.rearrange("(o p) m -> p o m", p=P)
    out_v = out_d.rearrange("(o p) m -> p o m", p=P)

    with tile.TileContext(nc) as tc:
        with (
            tc.tile_pool(name="cpool", bufs=1) as cpool,
            tc.tile_pool(name="bigs", bufs=1) as bigs,
            tc.tile_pool(name="hpool", bufs=1) as hpool,
            tc.tile_pool(name="epool", bufs=5) as epool,
            tc.tile_pool(name="zpool", bufs=2) as zpool,
            tc.tile_pool(name="spool", bufs=1) as spool,
            tc.tile_pool(name="wpool", bufs=2) as wpool,
            tc.tile_pool(name="psA", bufs=4, space="PSUM") as psA,
            tc.tile_pool(name="psO", bufs=4, space="PSUM") as psO,
        ):
            # ---- input loads: x split into 8 chunks so stats/warmup overlap ----
            x_sb = bigs.tile([P, CO, N], F32)
            x_dmas = []
            for co in range(CO):
                for c in range(4):
                    csl = slice(c * 1024, (c + 1) * 1024)
                    x_dmas.append(
                        nc.sync.dma_start(out=x_sb[:, co, csl], in_=x_v[:, co, csl])
                    )
            wqk_sb = cpool.tile([P, CO, 3 * C], F32R)
            w_dma = nc.sync.dma_start(out=wqk_sb, in_=wqkT_v)
            projT_sb = cpool.tile([P, CO, C], F32R)
            nc.sync.dma_start(out=projT_sb, in_=projT_v)

            def vec_tile(name, d_ap):
                t = cpool.tile([P, CO], F32, name=name)
                nc.sync.dma_start(out=t, in_=d_ap.rearrange("(o p) -> p o", p=P))
                return t

            bq_sb = vec_tile("bq_sb", bq_d)
            bk_sb = vec_tile("bk_sb", bk_d)
            pb_sb = vec_tile("pb_sb", pb_d)
            nw_sb = vec_tile("nw_sb", nw_d)
            nb_sb = vec_tile("nb_sb", nb_d)
            bv_row = cpool.tile([1, C], F32R)
            nc.sync.dma_start(out=bv_row, in_=bv_d.unsqueeze(0))
            gh_sb = cpool.tile([P, P], F32)
            nc.sync.dma_start(out=gh_sb, in_=gh_d)
            ones128 = cpool.tile([P, 1], F32R)
            nc.sync.dma_start(out=ones128, in_=ones128_d)
            ones1 = cpool.tile([1, P], F32R)
            nc.sync.dma_start(out=ones1, in_=ones1_d)
            eps_t = cpool.tile([P, 1], F32)
            nc.vector.memset(eps_t, EPS)

            # ---- GroupNorm stats (per-channel along free axis) ----
            stats = spool.tile([P, CO, 8, 6], F32)
            mv = spool.tile([P, CO, 2], F32)
            for co in range(CO):
                for s in range(8):
                    nc.vector.bn_stats(
                        out=stats[:, co, s, :],
                        in_=x_sb[:, co, s * 512 : (s + 1) * 512],
                    )
                nc.vector.bn_aggr(out=mv[:, co, :], in_=stats[:, co])
            # rstats cols: [mean_co0, mean_co1, ex2_co0, ex2_co1]
            rstats = spool.tile([P, 4], F32)
            nc.vector.tensor_copy(out=rstats[:, 0:2], in_=mv[:, :, 0])
            nc.vector.tensor_tensor(
                out=rstats[:, 2:4], in0=mv[:, :, 0], in1=mv[:, :, 0], op=AX.mult)
            nc.vector.tensor_tensor(
                out=rstats[:, 2:4], in0=rstats[:, 2:4], in1=mv[:, :, 1], op=AX.add)
            # group mean over 8 adjacent partitions, broadcast back, in one
            # block-diagonal (1/8) indicator matmul (fp32 exact)
            bps = psA.tile([P, 4], F32, tag="ps512", name="bps")
            nc.tensor.matmul(bps, lhsT=gh_sb, rhs=rstats, start=True, stop=True)
            bss = spool.tile([P, 4], F32)
            nc.vector.tensor_copy(out=bss, in_=bps)
            # var = ex2 - mu^2 ; rstd = 1/sqrt(var + eps)
            var = spool.tile([P, 2], F32)
            nc.vector.tensor_tensor(out=var, in0=bss[:, 0:2], in1=bss[:, 0:2], op=AX.mult)
            nc.vector.tensor_tensor(out=var, in0=bss[:, 2:4], in1=var, op=AX.subtract)
            sd = spool.tile([P, 2], F32)
            nc.scalar.activation(out=sd, in_=var, func=AF.Sqrt, bias=eps_t, scale=1.0)
            rstd = spool.tile([P, 2], F32)
            nc.vector.reciprocal(out=rstd, in_=sd)
            alpha = spool.tile([P, 2], F32)
            nc.vector.tensor_tensor(out=alpha, in0=rstd, in1=nw_sb, op=AX.mult)
            beta = spool.tile([P, 2], F32)
            nc.vector.tensor_tensor(out=beta, in0=bss[:, 0:2], in1=alpha, op=AX.mult)
            nc.vector.tensor_tensor(out=beta, in0=nb_sb, in1=beta, op=AX.subtract)

            # residual pre-fill: out <- x; proj results are DMA-accumulated later
            for co in range(CO):
                nc.sync.dma_start(out=out_v[:, co, :], in_=x_sb[:, co, :])

            # broadcast V bias row across partitions once: bvb[p, c] = bv[c]
            bvb_ps = psA.tile([P, C], F32, tag="ps512", name="bvb_ps")
            nc.tensor.matmul(bvb_ps, lhsT=ones1, rhs=bv_row, start=True, stop=True)
            bvb = cpool.tile([P, C], F32)
            nc.vector.tensor_copy(out=bvb, in_=bvb_ps)

            # ---- h + QKV, interleaved per 512-token block ----
            hn_sb = hpool.tile([P, CO, N], F32R, tag="hout", name="hn_sb")
            q_sb = bigs.tile([P, CO, N], F32R)
            k_sb = bigs.tile([P, CO, N], F32R)
            vt_sb = bigs.tile([P, NKC, C], F32R)

            for blk in range(NQB):
                sl = slice(blk * QB, (blk + 1) * QB)
                for co in range(CO):
                    nc.vector.tensor_scalar(
                        out=hn_sb[:, co, sl], in0=x_sb[:, co, sl],
                        scalar1=alpha[:, co : co + 1], scalar2=beta[:, co : co + 1],
                        op0=AX.mult, op1=AX.add,
                    )
                for dst, bias_sb, off in ((q_sb, bq_sb, 0), (k_sb, bk_sb, C)):
                    for cout in range(CO):
                        ps = psA.tile([P, QB], F32, tag="ps512", name="qk_ps")
                        for ci in range(CO):
                            nc.tensor.matmul(
                                ps,
                                lhsT=wqk_sb[:, ci, off + cout * P : off + (cout + 1) * P],
                                rhs=hn_sb[:, ci, sl],
                                start=(ci == 0), stop=(ci == CO - 1),
                            )
                        nc.scalar.activation(
                            out=dst[:, cout, sl], in_=ps,
                            func=AF.Identity, bias=bias_sb[:, cout : cout + 1], scale=1.0,
                        )
                for ko in range(4 * blk, 4 * blk + 4):
                    ps = psA.tile([P, C], F32, tag="ps512", name="vt_ps")
                    for ci in range(CO):
                        nc.tensor.matmul(
                            ps,
                            lhsT=hn_sb[:, ci, ko * P : (ko + 1) * P],
                            rhs=wqk_sb[:, ci, 2 * C : 3 * C],
                            start=(ci == 0), stop=(ci == CO - 1),
                        )
                    nc.vector.tensor_tensor(out=vt_sb[:, ko, :], in0=ps, in1=bvb, op=AX.add)

            # ---- attention (pipelined; prev block epilogue + proj injected) ----
            out_sb = hpool.tile([P, CO, N], F32R, tag="hout", name="out_sb")

            def make_block(qb):
                ctx = {"qb": qb}
                ctx["zacc"] = zpool.tile([P, QB], F32R, name="zacc")
                ctx["pso"] = [
                    psO.tile([P, QB], F32, tag="psout", name=f"pso{cc}")
                    for cc in range(CO)
                ]
                ctx["es"] = [None] * NKC
                return ctx

            def do_s(ctx, i, defer_z=False):
                qb = ctx["qb"]
                ps = psA.tile([P, QB], F32, tag="ps512", name="s_ps")
                for ci in range(CO):
                    nc.tensor.matmul(
                        ps,
                        lhsT=k_sb[:, ci, i * P : (i + 1) * P],
                        rhs=q_sb[:, ci, qb * QB : (qb + 1) * QB],
                        start=(ci == 0), stop=(ci == CO - 1),
                    )
                e = epool.tile([P, QB], F32R, name="e_tile")
                nc.scalar.activation(out=e, in_=ps, func=AF.Exp)
                ctx["es"][i] = e
                if not defer_z:
                    do_zadd(ctx, i)

            def do_zadd(ctx, i):
                if i == 1:
                    nc.vector.tensor_tensor(
                        out=ctx["zacc"], in0=ctx["es"][0], in1=ctx["es"][1], op=AX.add
                    )
                elif i > 1:
                    nc.vector.tensor_tensor(
                        out=ctx["zacc"], in0=ctx["zacc"], in1=ctx["es"][i], op=AX.add
                    )  # fp32r accumulate: ~1.7e-4 worst-case on Z, fine here

            def do_pv(ctx, i):
                for cc in range(CO):
                    nc.tensor.matmul(
                        ctx["pso"][cc],
                        lhsT=vt_sb[:, i, cc * P : (cc + 1) * P],
                        rhs=ctx["es"][i],
                        start=(i == 0), stop=(i == NKC - 1),
                    )

            def epi_zsum(ctx):
                zps = psA.tile([1, QB], F32, tag="ps512", name="zps")
                nc.tensor.matmul(zps, lhsT=ones128, rhs=ctx["zacc"], start=True, stop=True)
                zr = wpool.tile([1, QB], F32R, name="zr")
                with nc.allow_low_precision(reason="1/Z rounded to fp22 once"):
                    nc.vector.reciprocal(out=zr, in_=zps)
                ctx["zr"] = zr

            def epi_zb(ctx):
                zbp = psA.tile([P, QB], F32, tag="ps512", name="zbp")
                nc.tensor.matmul(zbp, lhsT=ones1, rhs=ctx["zr"], start=True, stop=True)
                zbs = wpool.tile([P, QB], F32, name="zbs")
                nc.scalar.copy(out=zbs, in_=zbp)
                ctx["zbs"] = zbs

            def epi_out(ctx):
                qb = ctx["qb"]
                for cc in range(CO):
                    nc.vector.tensor_tensor(
                        out=out_sb[:, cc, qb * QB : (qb + 1) * QB],
                        in0=ctx["pso"][cc], in1=ctx["zbs"], op=AX.mult,
                    )

            def epi_proj(ctx, cout, last=False):
                qb = ctx["qb"]
                sl = slice(qb * QB, (qb + 1) * QB)
                # psO pair slots of block qb-1 are free once epi_out ran (step
                # 9); proj is injected at steps 12/15 so it can use them,
                # keeping the S-chunk PSUM rotation in psA undisturbed
                ps = psO.tile([P, QB], F32, tag="psout", name="pj_ps")
                for ci in range(CO):
                    nc.tensor.matmul(
                        ps,
                        lhsT=projT_sb[:, ci, cout * P : (cout + 1) * P],
                        rhs=out_sb[:, ci, sl],
                        start=(ci == 0), stop=(ci == CO - 1),
                    )
                fin = wpool.tile([P, QB], F32, name="fin")
                if last:
                    nc.vector.scalar_tensor_tensor(
                        out=fin, in0=ps, scalar=pb_sb[:, cout : cout + 1],
                        in1=x_sb[:, cout, sl], op0=AX.add, op1=AX.add,
                    )
                    nc.sync.dma_start(out=out_v[:, cout, sl], in_=fin)
                else:
                    nc.scalar.activation(
                        out=fin, in_=ps, func=AF.Identity,
                        bias=pb_sb[:, cout : cout + 1], scale=1.0,
                    )
                    nc.gpsimd.dma_start(
                        out=out_v[:, cout, sl], in_=fin, accum_op=AX.add
                    )

            def inject(prev, step):
                if prev is None:
                    return
                if step == 3:
                    epi_zsum(prev)
                elif step == 6:
                    epi_zb(prev)
                elif step == 9:
                    epi_out(prev)
                elif step == 12:
                    epi_proj(prev, 0)
                elif step == 15:
                    epi_proj(prev, 1)

            prev = None
            for qb in range(NQB):
                ctx = make_block(qb)
                do_s(ctx, 0)
                do_s(ctx, 1, defer_z=True)
                do_s(ctx, 2, defer_z=True)
                for i in range(3, NKC):
                    do_pv(ctx, i - 3)
                    do_s(ctx, i, defer_z=(i == 3))
                    inject(prev, i)
                    if i == 3:
                        for j in (1, 2, 3):
                            do_zadd(ctx, j)
                do_pv(ctx, NKC - 3)
                do_pv(ctx, NKC - 2)
                do_pv(ctx, NKC - 1)
                prev = ctx
            # tail: last block epilogue + proj
            epi_zsum(prev)
            epi_zb(prev)
            epi_out(prev)
            epi_proj(prev, 0, last=True)
            epi_proj(prev, 1, last=True)

    nc.compile()
    return nc


def _host_inputs(x, norm_w, norm_b, qkv_w, qkv_b, proj_w, proj_b):
    f = np.float32
    scale = f(1.0) / f(16.0)
    wqkT = np.ascontiguousarray(qkv_w.T).astype(f)   # [c_in, 3C]
    wqkT[:, :C] *= scale
    bq = (qkv_b[:C] * scale).astype(f)
    bk = qkv_b[C : 2 * C].astype(f)
    bv = qkv_b[2 * C : 3 * C].astype(f)
    projT = np.ascontiguousarray(proj_w.T).astype(f)
    gh = np.zeros((P, P), f)
    gh[np.arange(P)[:, None] // 8 == np.arange(P)[None, :] // 8] = 0.125
    shared = {
        "wqkT": wqkT, "bq": bq, "bk": bk, "bv": bv,
        "projT": projT, "pb": proj_b.astype(f),
        "nw": norm_w.astype(f), "nb": norm_b.astype(f),
        "ghmat": gh,
        "ones128": np.ones((P, 1), f), "ones1": np.ones((1, P), f),
    }
    xs = np.ascontiguousarray(x.reshape(x.shape[0], C, N).astype(f))
    return [dict(shared, x=xs[i]) for i in range(x.shape[0])]


def kernel(x, norm_w, norm_b, qkv_w, qkv_b, proj_w, proj_b):
    global _LAST_RESULTS
    B = x.shape[0]
    nc = _build_program()
    in_maps = _host_inputs(x, norm_w, norm_b, qkv_w, qkv_b, proj_w, proj_b)
    trace = bool(int(os.environ.get("KERNEL_TRACE", "0"))) or bool(
        os.environ.get("BASS_TRACE")
    )
    if trace:
        trace = _ensure_ntff_hook()
    res = run_bass_kernel_spmd(
        nc, in_maps, core_ids=list(range(B)), trace=trace,
    )
    _LAST_RESULTS = res
    out = np.stack([res.results[i]["out"] for i in range(B)])
    return out.reshape(B, C, 64, 64)



# revision 7
# speedup vs baseline: 1.1604x; 1.1604x over previous
"""AttnBlock (GroupNorm -> 1x1 qkv conv -> full attention -> 1x1 proj -> residual)
for x[8, 256, 64, 64] fp32, data-parallel over batch on 8 NeuronCores.

v2: fp8e4m3 DoubleRow matmuls for QKV, scores and PV (2x PE throughput), with
the algebra folded so fp8 never touches the residual path:
  - GroupNorm folded into the conv weights on device: w8 = fp8(wqkT * alpha),
    x8 = fp8(x); the beta corrections are tiny on-device matmuls
    (q-bias += Wq@beta, k-bias cancels in softmax, v-bias -> proj bias).
  - Host folds: pb' = proj_b + proj_w @ bv (since sum(attn)=1); bk dropped
    (per-query constant cancels in softmax).
  - Scores stay at natural scale (sigma~16); exp on ACT applies
    scale=1/16, bias=-4 and writes fp8 e tiles that feed DoubleRow PV
    directly. exp(s/16-4) <= e^4.1 ~ 60 < 240 (fp8e4 max), no row max needed.
  - Z = sum_k e via a pipelined DVE/Pool bf16 add-tree over the fp8 e tiles
    (partition reduce by a ones-matmul), 1/Z via reciprocal_approx_fast,
    broadcast by a ones-matmul; epilogue of block b injected into block b+1's
    instruction streams so no engine stalls.
  - PSUM: 2x [P,2,512] score tiles (4 banks) + 4x [P,512] PV accumulators
    (2 generations) = 8 banks; zsum/zbroadcast borrow score slots briefly.
  - Residual: out DRAM prefilled with x, proj results DMA-accumulated
    (gpsimd accum_op=add); last block uses a fused DVE add + plain DMA.
"""

import contextlib
import ctypes
import os
import sys
import types

import numpy as np

import concourse.tile as tile
from concourse import bacc, mybir
from concourse.bass_utils import run_bass_kernel_spmd


def _ensure_ntff_hook() -> bool:
    """Install an antenv.axon_hooks shim backed by libaxon_pjrt.so so that
    run_bass_kernel_spmd(trace=True) can capture NTFF profiles under axon.
    Returns True when tracing is possible."""
    try:
        from antenv.axon_hooks import get_axon_ntff_profile_hook  # noqa: F401

        return True
    except ImportError:
        pass
    so_path = "/opt/axon/libaxon_pjrt.so"
    if not os.path.exists(so_path):
        return False
    try:
        lib = ctypes.CDLL(so_path)
        if not hasattr(lib, "axon_start_nrt_profile"):
            return False
        lib.axon_start_nrt_profile.argtypes = [
            ctypes.POINTER(ctypes.c_int64),
            ctypes.c_size_t,
        ]
        lib.axon_start_nrt_profile.restype = ctypes.c_int64
        lib.axon_stop_nrt_profile.argtypes = [ctypes.c_char_p]
        lib.axon_stop_nrt_profile.restype = ctypes.c_int64
    except OSError:
        return False

    @contextlib.contextmanager
    def _hook(output_dir, device_ids):
        import jax

        jax.devices()
        if device_ids:
            ids = (ctypes.c_int64 * len(device_ids))(*device_ids)
            rc = lib.axon_start_nrt_profile(ids, len(device_ids))
        else:
            rc = lib.axon_start_nrt_profile(None, 0)
        if rc != 0:
            raise RuntimeError(f"axon_start_nrt_profile rc={rc}")
        try:
            yield
        finally:
            n = lib.axon_stop_nrt_profile(str(output_dir).encode())
            print(f"profile: {n} file(s) written to {output_dir}", file=sys.stderr)

    mod = types.ModuleType("antenv.axon_hooks")
    _state = {"hook": _hook}
    mod.get_axon_ntff_profile_hook = lambda: _state["hook"]
    mod.set_axon_ntff_profile_hook = lambda h: _state.__setitem__("hook", h)
    sys.modules["antenv.axon_hooks"] = mod
    import antenv

    antenv.axon_hooks = mod
    return True

F32 = mybir.dt.float32
F32R = mybir.dt.float32r
BF16 = mybir.dt.bfloat16
F8 = mybir.dt.float8e4
AX = mybir.AluOpType
AF = mybir.ActivationFunctionType
DR = mybir.MatmulPerfMode.DoubleRow

C = 256          # channels
N = 4096         # tokens (64*64)
P = 128          # partitions
CO = 2           # channel chunks (C // P)
QB = 512         # queries per block
NQB = N // QB    # 8 query blocks
NKC = N // P     # 32 key chunks
NPR = NKC // 2   # 16 key chunk pairs (DoubleRow contracts 256 keys)
EPS = 1e-5

_LAST_RESULTS = None


def _build_program():
    nc = bacc.Bacc("TRN2", target_bir_lowering=False, debug=False, num_devices=8)

    x_d = nc.dram_tensor("x", [C, N], F32, kind="ExternalInput").ap()
    wqkT_d = nc.dram_tensor("wqkT", [C, 3 * C], F32, kind="ExternalInput").ap()
    bq_d = nc.dram_tensor("bq", [C], F32, kind="ExternalInput").ap()
    projT_d = nc.dram_tensor("projT", [C, C], F32, kind="ExternalInput").ap()
    pb_d = nc.dram_tensor("pb", [C], F32, kind="ExternalInput").ap()
    nw_d = nc.dram_tensor("nw", [C], F32, kind="ExternalInput").ap()
    nb_d = nc.dram_tensor("nb", [C], F32, kind="ExternalInput").ap()
    gh_d = nc.dram_tensor("ghmat", [P, P], F32, kind="ExternalInput").ap()
    out_d = nc.dram_tensor("out", [C, N], F32, kind="ExternalOutput").ap()

    # channel c = o*128 + p  ->  [partition, chunk, free]
    x_v = x_d.rearrange("(o p) m -> p o m", p=P)
    wqkT_v = wqkT_d.rearrange("(o p) m -> p o m", p=P)
    projT_v = projT_d.rearrange("(o p) m -> p o m", p=P)
    out_v = out_d.rearrange("(o p) m -> p o m", p=P)

    with tile.TileContext(nc) as tc:
        with (
            tc.tile_pool(name="cpool", bufs=1) as cpool,
            tc.tile_pool(name="bigs", bufs=1) as bigs,
            tc.tile_pool(name="spool", bufs=1) as spool,
            tc.tile_pool(name="epool", bufs=6) as epool,
            tc.tile_pool(name="t1pool", bufs=4) as t1pool,
            tc.tile_pool(name="t2pool", bufs=3) as t2pool,
            tc.tile_pool(name="t3pool", bufs=2) as t3pool,
            tc.tile_pool(name="zpool", bufs=2) as zpool,
            tc.tile_pool(name="wpool", bufs=2) as wpool,
            tc.tile_pool(name="psA", bufs=2, space="PSUM") as psA,
            tc.tile_pool(name="psO", bufs=4, space="PSUM") as psO,
        ):
            # ---- input loads: x split into 8 chunks so stats/casts overlap ----
            x_sb = bigs.tile([P, CO, N], F32)
            x8 = bigs.tile([P, CO, N], F8)
            for co in range(CO):
                for c in range(4):
                    csl = slice(c * 1024, (c + 1) * 1024)
                    nc.sync.dma_start(out=x_sb[:, co, csl], in_=x_v[:, co, csl])
            wqk_sb = cpool.tile([P, CO, 3 * C], F32)
            nc.sync.dma_start(out=wqk_sb, in_=wqkT_v)
            projT_sb = cpool.tile([P, CO, C], F32)
            nc.sync.dma_start(out=projT_sb, in_=projT_v)

            def vec_tile(name, d_ap):
                t = cpool.tile([P, CO], F32, name=name)
                nc.sync.dma_start(out=t, in_=d_ap.rearrange("(o p) -> p o", p=P))
                return t

            bq_sb = vec_tile("bq_sb", bq_d)
            pb_sb = vec_tile("pb_sb", pb_d)
            nw_sb = vec_tile("nw_sb", nw_d)
            nb_sb = vec_tile("nb_sb", nb_d)
            gh_sb = cpool.tile([P, P], F32)
            nc.sync.dma_start(out=gh_sb, in_=gh_d)
            ones1b = cpool.tile([1, P], BF16)
            nc.vector.memset(ones1b, 1.0)
            ones128b = cpool.tile([P, 1], BF16)
            nc.vector.memset(ones128b, 1.0)
            eps_t = cpool.tile([P, 1], F32)
            nc.vector.memset(eps_t, EPS)
            neg4_t = cpool.tile([P, 1], F32)
            nc.vector.memset(neg4_t, -4.0)

            # x8 casts (ACT) chunk-by-chunk, hidden under the x DMA
            for co in range(CO):
                for c in range(4):
                    csl = slice(c * 1024, (c + 1) * 1024)
                    nc.scalar.copy(out=x8[:, co, csl], in_=x_sb[:, co, csl])

            # ---- GroupNorm stats (per-channel along free axis) ----
            stats = spool.tile([P, CO, 8, 6], F32)
            mv = spool.tile([P, CO, 2], F32)
            for co in range(CO):
                for s in range(8):
                    nc.vector.bn_stats(
                        out=stats[:, co, s, :],
                        in_=x_sb[:, co, s * 512 : (s + 1) * 512],
                    )
                nc.vector.bn_aggr(out=mv[:, co, :], in_=stats[:, co])
            # rstats cols: [mean_co0, mean_co1, ex2_co0, ex2_co1]
            rstats = spool.tile([P, 4], F32)
            nc.vector.tensor_copy(out=rstats[:, 0:2], in_=mv[:, :, 0])
            nc.vector.tensor_tensor(
                out=rstats[:, 2:4], in0=mv[:, :, 0], in1=mv[:, :, 0], op=AX.mult)
            nc.vector.tensor_tensor(
                out=rstats[:, 2:4], in0=rstats[:, 2:4], in1=mv[:, :, 1], op=AX.add)
            # group mean over 8 adjacent partitions, broadcast back, in one
            # block-diagonal (1/8) indicator matmul (fp32 exact)
            bps = psA.tile([P, 4], F32, tag="spair", name="bps")
            nc.tensor.matmul(bps, lhsT=gh_sb, rhs=rstats, start=True, stop=True)
            bss = spool.tile([P, 4], F32)
            nc.vector.tensor_copy(out=bss, in_=bps)
            # var = ex2 - mu^2 ; rstd = 1/sqrt(var + eps)
            var = spool.tile([P, 2], F32)
            nc.vector.tensor_tensor(out=var, in0=bss[:, 0:2], in1=bss[:, 0:2], op=AX.mult)
            nc.vector.tensor_tensor(out=var, in0=bss[:, 2:4], in1=var, op=AX.subtract)
            sd = spool.tile([P, 2], F32)
            nc.scalar.activation(out=sd, in_=var, func=AF.Sqrt, bias=eps_t, scale=1.0)
            rstd = spool.tile([P, 2], F32)
            nc.vector.reciprocal(out=rstd, in_=sd)
            alpha = spool.tile([P, 2], F32)
            nc.vector.tensor_tensor(out=alpha, in0=rstd, in1=nw_sb, op=AX.mult)
            beta = spool.tile([P, 2], F32)
            nc.vector.tensor_tensor(out=beta, in0=bss[:, 0:2], in1=alpha, op=AX.mult)
            nc.vector.tensor_tensor(out=beta, in0=nb_sb, in1=beta, op=AX.subtract)

            # residual pre-fill: out <- x; proj results are DMA-accumulated later
            for co in range(CO):
                nc.sync.dma_start(out=out_v[:, co, :], in_=x_sb[:, co, :])

            with nc.allow_low_precision(reason="fp8 attention path"):
                # ---- fold groupnorm scale into the conv weights ----
                wqk8 = cpool.tile([P, CO, 3 * C], F8)
                for co in range(CO):
                    nc.vector.tensor_scalar(
                        out=wqk8[:, co, :], in0=wqk_sb[:, co, :],
                        scalar1=alpha[:, co : co + 1], scalar2=None, op0=AX.mult)
                projT8 = cpool.tile([P, CO, C], F8)
                nc.vector.tensor_copy(out=projT8, in_=projT_sb)

                # q bias correction: bqt = bq + Wq @ beta  (per q-channel)
                # v bias correction: cv = Wv @ beta -> pbt = pb + projW @ cv
                bias_ps = psA.tile([P, 4], F32, tag="spair", name="bias_ps")
                for cout in range(CO):
                    for ci in range(CO):
                        nc.tensor.matmul(
                            bias_ps[:, cout : cout + 1],
                            lhsT=wqk_sb[:, ci, cout * P : (cout + 1) * P],
                            rhs=beta[:, ci : ci + 1],
                            start=(ci == 0), stop=(ci == CO - 1))
                for cout in range(CO):
                    for ci in range(CO):
                        nc.tensor.matmul(
                            bias_ps[:, 2 + cout : 3 + cout],
                            lhsT=wqk_sb[:, ci, 2 * C + cout * P : 2 * C + (cout + 1) * P],
                            rhs=beta[:, ci : ci + 1],
                            start=(ci == 0), stop=(ci == CO - 1))
                bqt = spool.tile([P, CO], F32)
                nc.vector.tensor_tensor(
                    out=bqt, in0=bq_sb, in1=bias_ps[:, 0:2], op=AX.add)
                cv_sb = spool.tile([P, CO], F32)
                nc.vector.tensor_copy(out=cv_sb, in_=bias_ps[:, 2:4])
                pv_ps = psA.tile([P, 2], F32, tag="spair", name="pv_ps")
                for cout in range(CO):
                    for ci in range(CO):
                        nc.tensor.matmul(
                            pv_ps[:, cout : cout + 1],
                            lhsT=projT_sb[:, ci, cout * P : (cout + 1) * P],
                            rhs=cv_sb[:, ci : ci + 1],
                            start=(ci == 0), stop=(ci == CO - 1))
                pbt = spool.tile([P, CO], F32)
                nc.vector.tensor_tensor(
                    out=pbt, in0=pb_sb, in1=pv_ps, op=AX.add)

                # ---- QKV (DoubleRow fp8), k/v bias-free by softmax algebra ----
                q8 = bigs.tile([P, CO, N], F8)
                k8 = bigs.tile([P, CO, N], F8)
                vt8 = bigs.tile([P, NKC, C], F8)
                out8 = bigs.tile([P, CO, N], F8)

                for blk in range(NQB):
                    sl = slice(blk * QB, (blk + 1) * QB)
                    qk_ps = psA.tile([P, 2, QB], F32, tag="spair", name="qk_ps")
                    for cout in range(CO):
                        nc.tensor.matmul(
                            qk_ps[:, cout, :],
                            lhsT=wqk8[:, :, cout * P : (cout + 1) * P],
                            rhs=x8[:, :, sl],
                            start=True, stop=True, perf_mode=DR)
                        # q cast with bias rides ACT; k plain cast on DVE below
                        nc.scalar.activation(
                            out=q8[:, cout, sl], in_=qk_ps[:, cout, :],
                            func=AF.Identity, bias=bqt[:, cout : cout + 1], scale=1.0)
                    kk_ps = psA.tile([P, 2, QB], F32, tag="spair", name="kk_ps")
                    for cout in range(CO):
                        nc.tensor.matmul(
                            kk_ps[:, cout, :],
                            lhsT=wqk8[:, :, C + cout * P : C + (cout + 1) * P],
                            rhs=x8[:, :, sl],
                            start=True, stop=True, perf_mode=DR)
                        nc.vector.tensor_copy(out=k8[:, cout, sl], in_=kk_ps[:, cout, :])
                    # vt pairs: 4 token chunks -> 2 psum pair tiles
                    for kp in range(2 * blk, 2 * blk + 2):
                        vt_ps = psA.tile([P, 2, QB], F32, tag="spair", name="vt_ps")
                        for i in range(2):
                            ko = 2 * kp + i
                            nc.tensor.matmul(
                                vt_ps[:, i, 0:C],
                                lhsT=x8[:, :, ko * P : (ko + 1) * P],
                                rhs=wqk8[:, :, 2 * C : 3 * C],
                                start=True, stop=True, perf_mode=DR)
                        eng = nc.vector if kp % 2 == 0 else nc.scalar
                        if eng is nc.vector:
                            nc.vector.tensor_copy(
                                out=vt8[:, 2 * kp : 2 * kp + 2, :], in_=vt_ps[:, :, 0:C])
                        else:
                            nc.scalar.copy(
                                out=vt8[:, 2 * kp : 2 * kp + 2, :], in_=vt_ps[:, :, 0:C])

                # ---- attention (pipelined; prev block epilogue injected) ----
                def make_block(qb):
                    ctx = {"qb": qb}
                    ctx["pso"] = [
                        psO.tile([P, QB], F32, tag="psout", name=f"pso{cc}")
                        for cc in range(CO)
                    ]
                    ctx["es"] = [None] * NPR
                    ctx["t1"] = [None] * 8
                    ctx["t2"] = [None] * 4
                    ctx["t3"] = [None] * 2
                    return ctx

                def do_s(ctx, j):
                    qb = ctx["qb"]
                    ps = psA.tile([P, 2, QB], F32, tag="spair", name="s_ps")
                    for i in range(2):
                        kc = 2 * j + i
                        nc.tensor.matmul(
                            ps[:, i, :],
                            lhsT=k8[:, :, kc * P : (kc + 1) * P],
                            rhs=q8[:, :, qb * QB : (qb + 1) * QB],
                            start=True, stop=True, perf_mode=DR)
                    e = epool.tile([P, 2, QB], F8, name="e_tile")
                    nc.scalar.activation(
                        out=e, in_=ps, func=AF.Exp, bias=neg4_t, scale=1.0 / 16.0)
                    ctx["es"][j] = e

                def do_tree(ctx, j):
                    # after pair j completes, emit any tree ops whose inputs
                    # are ready: t1[i] at j=2i+1, t2[i] at j=4i+3, ...
                    if j % 2 == 1:
                        i = j // 2
                        eng = nc.gpsimd if i < 4 else nc.vector
                        t = t1pool.tile([P, 2, QB], BF16, name="t1")
                        eng.tensor_tensor(
                            out=t, in0=ctx["es"][2 * i], in1=ctx["es"][2 * i + 1],
                            op=AX.add)
                        ctx["t1"][i] = t
                    if j % 4 == 3:
                        i = j // 4
                        t = t2pool.tile([P, 2, QB], BF16, name="t2")
                        nc.vector.tensor_tensor(
                            out=t, in0=ctx["t1"][2 * i], in1=ctx["t1"][2 * i + 1],
                            op=AX.add)
                        ctx["t2"][i] = t
                    if j % 8 == 7:
                        i = j // 8
                        t = t3pool.tile([P, 2, QB], BF16, name="t3")
                        nc.vector.tensor_tensor(
                            out=t, in0=ctx["t2"][2 * i], in1=ctx["t2"][2 * i + 1],
                            op=AX.add)
                        ctx["t3"][i] = t
                    if j == NPR - 1:
                        z = zpool.tile([P, 2, QB], BF16, name="zacc")
                        nc.vector.tensor_tensor(
                            out=z, in0=ctx["t3"][0], in1=ctx["t3"][1], op=AX.add)
                        ctx["zacc"] = z

                def do_pv(ctx, j):
                    for cc in range(CO):
                        nc.tensor.matmul(
                            ctx["pso"][cc],
                            lhsT=vt8[:, 2 * j : 2 * j + 2, cc * P : (cc + 1) * P],
                            rhs=ctx["es"][j],
                            start=(j == 0), stop=(j == NPR - 1), perf_mode=DR)

                def epi_zfold(ctx):
                    z5 = wpool.tile([P, QB], BF16, name="z5")
                    nc.vector.tensor_tensor(
                        out=z5, in0=ctx["zacc"][:, 0, :], in1=ctx["zacc"][:, 1, :],
                        op=AX.add)
                    ctx["z5"] = z5

                def epi_zsum(ctx):
                    zps = psA.tile([P, 2, QB], F32, tag="spair", name="zps")
                    nc.tensor.matmul(
                        zps[0:1, 0, :], lhsT=ones128b, rhs=ctx["z5"],
                        start=True, stop=True)
                    ctx["zps"] = zps

                def epi_recip(ctx):
                    zr = wpool.tile([1, QB], F32, name="zr")
                    nc.vector.reciprocal_approx_fast(out=zr, in_=ctx["zps"][0:1, 0, :])
                    zrb = wpool.tile([1, QB], BF16, name="zrb")
                    nc.vector.tensor_copy(out=zrb, in_=zr)
                    ctx["zrb"] = zrb

                def epi_zb(ctx):
                    nc.tensor.matmul(
                        ctx["zps"][:, 1, :], lhsT=ones1b, rhs=ctx["zrb"],
                        start=True, stop=True)
                    zbs = wpool.tile([P, QB], F32, name="zbs")
                    nc.vector.tensor_copy(out=zbs, in_=ctx["zps"][:, 1, :])
                    ctx["zbs"] = zbs

                def epi_out(ctx, cc):
                    qb = ctx["qb"]
                    nc.vector.tensor_tensor(
                        out=out8[:, cc, qb * QB : (qb + 1) * QB],
                        in0=ctx["pso"][cc], in1=ctx["zbs"], op=AX.mult)

                def epi_proj(ctx, cout, last=False):
                    qb = ctx["qb"]
                    sl = slice(qb * QB, (qb + 1) * QB)
                    # proj psum borrows a score-pair slot (half per cout) so
                    # the psO pool stays a clean 2-generation PV rotation
                    if cout == 0:
                        ctx["pjps"] = psA.tile(
                            [P, 2, QB], F32, tag="spair", name="pj_ps")
                    ps = ctx["pjps"][:, cout, :]
                    nc.tensor.matmul(
                        ps,
                        lhsT=projT8[:, :, cout * P : (cout + 1) * P],
                        rhs=out8[:, :, sl],
                        start=True, stop=True, perf_mode=DR)
                    fin = wpool.tile([P, QB], F32, name="fin")
                    if last:
                        nc.vector.scalar_tensor_tensor(
                            out=fin, in0=ps, scalar=pbt[:, cout : cout + 1],
                            in1=x_sb[:, cout, sl], op0=AX.add, op1=AX.add)
                        nc.sync.dma_start(out=out_v[:, cout, sl], in_=fin)
                    else:
                        nc.vector.tensor_scalar(
                            out=fin, in0=ps, scalar1=pbt[:, cout : cout + 1],
                            scalar2=None, op0=AX.add)
                        nc.gpsimd.dma_start(
                            out=out_v[:, cout, sl], in_=fin, accum_op=AX.add)

                def inject(prev, j):
                    if prev is None:
                        return
                    if j == 1:
                        epi_zfold(prev)
                    elif j == 2:
                        epi_zsum(prev)
                    elif j == 3:
                        epi_recip(prev)
                    elif j == 4:
                        epi_zb(prev)
                    elif j == 5:
                        epi_out(prev, 0)
                    elif j == 6:
                        epi_out(prev, 1)
                    elif j == 8:
                        epi_proj(prev, 0)
                    elif j == 10:
                        epi_proj(prev, 1)

                prev = None
                for qb in range(NQB):
                    ctx = make_block(qb)
                    do_s(ctx, 0)
                    do_s(ctx, 1)
                    do_tree(ctx, 1)
                    for j in range(2, NPR):
                        do_pv(ctx, j - 2)
                        do_s(ctx, j)
                        inject(prev, j - 2)
                        do_tree(ctx, j)
                    do_pv(ctx, NPR - 2)
                    inject(prev, NPR - 2)
                    do_pv(ctx, NPR - 1)
                    inject(prev, NPR - 1)
                    prev = ctx
                # tail: last block epilogue
                epi_zfold(prev)
                epi_zsum(prev)
                epi_recip(prev)
                epi_zb(prev)
                epi_out(prev, 0)
                epi_out(prev, 1)
                epi_proj(prev, 0, last=True)
                epi_proj(prev, 1, last=True)

    nc.compile()
    return nc


def _host_inputs(x, norm_w, norm_b, qkv_w, qkv_b, proj_w, proj_b):
    f = np.float32
    wqkT = np.ascontiguousarray(qkv_w.T).astype(f)   # [c_in, 3C]
    bq = qkv_b[:C].astype(f)
    bv = qkv_b[2 * C : 3 * C].astype(f)
    projT = np.ascontiguousarray(proj_w.T).astype(f)
    # v bias folds into the proj bias because sum_k attn = 1
    pb = (proj_b + proj_w @ bv).astype(f)
    gh = np.zeros((P, P), f)
    gh[np.arange(P)[:, None] // 8 == np.arange(P)[None, :] // 8] = 0.125
    shared = {
        "wqkT": wqkT, "bq": bq,
        "projT": projT, "pb": pb,
        "nw": norm_w.astype(f), "nb": norm_b.astype(f),
        "ghmat": gh,
    }
    xs = np.ascontiguousarray(x.reshape(x.shape[0], C, N).astype(f))
    return [dict(shared, x=xs[i]) for i in range(x.shape[0])]


def kernel(x, norm_w, norm_b, qkv_w, qkv_b, proj_w, proj_b):
    global _LAST_RESULTS
    B = x.shape[0]
    nc = _build_program()
    in_maps = _host_inputs(x, norm_w, norm_b, qkv_w, qkv_b, proj_w, proj_b)
    trace = bool(int(os.environ.get("KERNEL_TRACE", "0"))) or bool(
        os.environ.get("BASS_TRACE")
    )
    if trace:
        trace = _ensure_ntff_hook()
    res = run_bass_kernel_spmd(
        nc, in_maps, core_ids=list(range(B)), trace=trace,
    )
    _LAST_RESULTS = res
    out = np.stack([res.results[i]["out"] for i in range(B)])
    return out.reshape(B, C, 64, 64)
